# revision 1
# baseline (speedup 1.0000x reference)
"""Trainium2 Bass kernel for nn_BinaryPathEncoder.

Math: the reference computes, for each position p, the ordered product of
rotation matrices along p's binary path (LSB-first, leading 1-bit stripped):
    R(p) = M_{b0} @ M_{b1} @ ... @ M_{b(k-1)},  M_b = expm(B_b - B_b^T)^T
Key identity: R(p) = M_{p&1} @ R(p>>1).  Splitting the <=16-step path into
6+5+5 bit chunks gives R(p) = R(idxA) @ R(idxB) @ R(idxC) with idxA<128,
idxB<64, idxC<64, so two small fp16 SBUF tables (natural R[q], q<64, and
transposed R[q]^T, q<128) cover every position with 2 matmuls:
  product1: X1T = matmul(lhsT=Rn[idxB](staged), rhs=Rt[idxA]) = (TA@TB)^T
  product2: O   = matmul(lhsT=X1T,              rhs=Rn[idxC]) = TA@TB@TC
Data-dependent entry selection uses host-computed per-core element offsets:
moving operands via register-offset APs on the PE (walrus allows register
offsets on the moving operand, not ldweights), the stationary operand staged
by a register-offset gpsimd copy.  One SPMD graph, per-core index data.
expm is computed on-device in f32 (scaling-and-squaring Taylor, s=4, n=8).
"""

import contextlib
import numpy as np

DIM = 256
NCORES = 8
P = 128

NAT_E = 63                     # natural table entries (q in [1,64))
TRA_E = 65                     # transposed entries: slot0=identity, slots 1..64 = q in [64,128)
ENT = 512                      # elements per partition per entry (2 kc x 256)
NAT_STRIDE = NAT_E * ENT
TRA_STRIDE = TRA_E * ENT

NSTAGE = 4                     # psum/output pipeline slots
NSTAGE_B = 8                   # lhsT staging slots (absorbs DMA latency)
NSX = 8                        # X1T staging slots (decouples cast from prod2 pace)
NOUT = 4                       # output buffer slots
EXPM_S = 4                     # scaling: A = skew / 2^s
EXPM_N = 8                     # Taylor (Horner) order

_NC_CACHE = {}
LAST_RESULTS = None


def _build_nc(npos, debug=False):
    from concourse import bass, bacc, mybir

    f32 = mybir.dt.float32
    f32r = mybir.dt.float32r
    f16 = mybir.dt.float16
    i32 = mybir.dt.int32
    Sub = mybir.AluOpType.subtract
    Add = mybir.AluOpType.add
    Eq = mybir.AluOpType.is_equal

    nc = bacc.Bacc("TRN2", target_bir_lowering=False, debug=debug)

    prims_ext = nc.dram_tensor("prims", [2, DIM, DIM], f32, kind="ExternalInput")
    assert npos % 2 == 0
    npair = npos // 2
    ncol = (npos + P - 1) // P
    ncolp = (npair + P - 1) // P
    offs_a_ext = nc.dram_tensor("offs_a", [P, ncolp], i32,
                                kind="ExternalInput")
    offs_c_ext = nc.dram_tensor("offs_c", [P, ncolp], i32,
                                kind="ExternalInput")
    offs_gp_ext = nc.dram_tensor("offs_gp", [P, ncol], i32,
                                 kind="ExternalInput")
    out_ext = nc.dram_tensor("out", [npos, 2, P, DIM], f16, kind="ExternalOutput")

    with contextlib.ExitStack() as ctx:
        sem = {}
        for name in (["in_sem", "pe_sem", "dve_sem", "act_sem", "gps_sem",
                      "mm1_sem", "mm2_sem", "dvex_sem"]
                     + [f"dma_s{j}" for j in range(NOUT)]
                     + [f"stg_s{j}" for j in range(NSTAGE_B // 2)]):
            sem[name] = ctx.enter_context(nc.semaphore(name))

        # ---- persistent SBUF ----
        rn = ctx.enter_context(nc.sbuf_tensor("rn", [P, NAT_STRIDE], f16))
        rt = ctx.enter_context(nc.sbuf_tensor("rt", [P, TRA_STRIDE], f16))
        offs_a = ctx.enter_context(nc.sbuf_tensor("offs_a_sb", [P, ncolp], i32))
        offs_c = ctx.enter_context(nc.sbuf_tensor("offs_c_sb", [P, ncolp], i32))
        offs_gp = ctx.enter_context(nc.sbuf_tensor("offs_gp_sb", [P, ncol], i32))
        pbf = ctx.enter_context(nc.sbuf_tensor("pbf", [P, 2, 2, DIM], f16))
        identf = ctx.enter_context(nc.sbuf_tensor("identf", [P, 2, DIM], f32))
        # ---- expm temporaries: scoped; their SBUF space is reused by the
        # position-phase staging buffers (runtime lifetimes are disjoint,
        # ordered by the build/position semaphore barrier) ----
        tmp_ctx = contextlib.ExitStack()
        prim = tmp_ctx.enter_context(nc.sbuf_tensor("prim", [P, 2, DIM], f32))
        an = tmp_ctx.enter_context(nc.sbuf_tensor("an", [P, 2, DIM], f32r))
        ap = tmp_ctx.enter_context(nc.sbuf_tensor("ap_", [P, 2, DIM], f32r))
        ytA = tmp_ctx.enter_context(nc.sbuf_tensor("ytA", [P, 2, DIM], f32r))
        ytB = tmp_ctx.enter_context(nc.sbuf_tensor("ytB", [P, 2, DIM], f32r))
        ci = tmp_ctx.enter_context(nc.sbuf_tensor("ci", [P, DIM], f32))
        pi = tmp_ctx.enter_context(nc.sbuf_tensor("pi", [P, 2], f32))
        tmp_ctx.close()
        stag_b = ctx.enter_context(nc.sbuf_tensor("stag_b", [P, NSTAGE_B, 2, DIM], f16))
        stag_x = ctx.enter_context(nc.sbuf_tensor("stag_x", [P, NSX, 2, DIM], f16))
        outb = ctx.enter_context(nc.sbuf_tensor("outb", [P, NOUT, 2, DIM], f16))
        # ---- PSUM: 8 banks ----
        ps = [ctx.enter_context(nc.psum_tensor(f"ps{j}", [P, 2, DIM], f32))
              for j in range(8)]

        ident128 = identf[:, 0, 0:P]

        def ent3(tab, q):
            """table entry q as a [P, 2, DIM] static AP"""
            if tab is rn:
                stride, slot = NAT_STRIDE, q - 1
            else:
                stride, slot = TRA_STRIDE, (0 if q == 1 else q - 63)
            return bass.AP(tab, slot * ENT, [[stride, P], [DIM, 2], [1, DIM]])

        cnt = {k: 0 for k in sem}
        entry_done = {}
        pe_prog, dve_prog, act_prog, gps_prog, sync_prog = [], [], [], [], []

        IN_TOTAL = 16 * 6

        # ---------------- DMA in (sync engine) ----------------
        def s_in(s):
            s.dma_start(offs_a[:, :], offs_a_ext[:, :]).then_inc(sem["in_sem"], 16)
            s.dma_start(offs_c[:, :], offs_c_ext[:, :]).then_inc(sem["in_sem"], 16)
            s.dma_start(offs_gp[:, :], offs_gp_ext[:, :]).then_inc(sem["in_sem"], 16)
            for kc in range(2):
                s.dma_start(prim[:, kc, :],
                            prims_ext[0, kc * P:(kc + 1) * P, :],
                            ).then_inc(sem["in_sem"], 16)
        sync_prog.append(s_in)
        cnt["in_sem"] = 16 * 5

        # ---------------- identity construction ----------------
        def g_iota(g):
            g.iota(ci[:, :], [[1, DIM]], channel_multiplier=0,
                   allow_small_or_imprecise_dtypes=True)
            g.iota(pi[:, 0:1], [[1, 1]], channel_multiplier=1,
                   allow_small_or_imprecise_dtypes=True)
            g.iota(pi[:, 1:2], [[1, 1]], base=P, channel_multiplier=1,
                   allow_small_or_imprecise_dtypes=True).then_inc(sem["gps_sem"], 1)
        gps_prog.append(g_iota)
        cnt["gps_sem"] += 1

        def d_ident(d, w=cnt["gps_sem"]):
            d.wait_ge(sem["gps_sem"], w)
            for kc in range(2):
                d.tensor_tensor(out=identf[:, kc, :], in0=ci[:, :],
                                in1=pi[:, kc:kc + 1].to_broadcast([P, DIM]), op=Eq)
            d.drain()
            d.tensor_copy(ent3(rn, 1), identf[:, :, :])
            d.tensor_copy(ent3(rt, 1), identf[:, :, :]).then_inc(sem["dve_sem"], 1)
        dve_prog.append(d_ident)
        cnt["dve_sem"] += 1
        ident_done = cnt["dve_sem"]

        # ---------------- expm for b = 0, 1 (f32r matmuls) ----------------
        # Double chain: E = expm(A/2^s) and ET = expm(-A/2^s) computed jointly;
        # squarings need no transposes since each chain is the other's lhsT:
        #   X <- matmul(lhsT=XT, rhs=X),  XT <- matmul(lhsT=X, rhs=XT)
        inv2s = 1.0 / (2.0 ** EXPM_S)

        for b in range(2):
            src = prim

            if b == 1:
                def s_in2(s, wpe=cnt["pe_sem"], wdve=cnt["dve_sem"]):
                    s.wait_ge(sem["pe_sem"], wpe)
                    s.wait_ge(sem["dve_sem"], wdve)
                    for kc in range(2):
                        s.dma_start(prim[:, kc, :],
                                    prims_ext[1, kc * P:(kc + 1) * P, :],
                                    ).then_inc(sem["in_sem"], 16)
                sync_prog.append(s_in2)
                cnt["in_sem"] += 32

            def p_tr(t, b=b, win=cnt["in_sem"], wid=ident_done):
                t.wait_ge(sem["in_sem"], win)
                if b == 0:
                    t.wait_ge(sem["dve_sem"], wid)
                last = None
                for kc in range(2):
                    for mc in range(2):
                        last = t.transpose(
                            out=ps[0][:, kc, mc * P:(mc + 1) * P],
                            in_=src[:, mc, kc * P:(kc + 1) * P],
                            identity=ident128)
                last.then_inc(sem["pe_sem"], 1)
            pe_prog.append(p_tr)
            cnt["pe_sem"] += 1

            def d_an(d, w=cnt["pe_sem"]):
                d.wait_ge(sem["pe_sem"], w)
                # an = -A = (B - B^T)/2^s is wrong sign; A = (B - B^T)/16:
                # ps0 holds B^T; an := (ps0 - B) * inv2s = -A
                d.tensor_tensor(out=an[:, :, :], in0=ps[0][:, :, :],
                                in1=prim[:, :, :], op=Sub)
                d.drain()
                d.tensor_scalar_mul(an[:, :, :], an[:, :, :], inv2s)
                d.drain()
                d.tensor_scalar_mul(ap[:, :, :], an[:, :, :], -1.0)
                d.tensor_scalar_mul(ytA[:, :, :], an[:, :, :], -1.0 / EXPM_N)
                d.drain()
                d.tensor_scalar_mul(ytB[:, :, :], ap[:, :, :],
                                    -1.0 / EXPM_N).then_inc(sem["dve_sem"], 1)
            dve_prog.append(d_an)
            cnt["dve_sem"] += 1

            # Horner: ytA_k = (A@ytA + A)/k ; ytB_k = (-A@ytB - A)/k
            first_h = True
            for k in range(EXPM_N - 1, 0, -1):
                def p_hA(t, w=cnt["dve_sem"] - (0 if first_h else 1)):
                    t.wait_ge(sem["dve_sem"], w)
                    last = None
                    for mc in range(2):
                        for kc in range(2):
                            last = t.matmul(ps[1][:, mc, :],
                                            an[:, kc, mc * P:(mc + 1) * P],
                                            ytA[:, kc, :],
                                            start=(kc == 0), stop=(kc == 1))
                    last.then_inc(sem["pe_sem"], 1)
                pe_prog.append(p_hA)
                cnt["pe_sem"] += 1

                def p_hB(t, w=cnt["dve_sem"]):
                    t.wait_ge(sem["dve_sem"], w)
                    last = None
                    for mc in range(2):
                        for kc in range(2):
                            last = t.matmul(ps[2][:, mc, :],
                                            ap[:, kc, mc * P:(mc + 1) * P],
                                            ytB[:, kc, :],
                                            start=(kc == 0), stop=(kc == 1))
                    last.then_inc(sem["pe_sem"], 1)
                pe_prog.append(p_hB)
                cnt["pe_sem"] += 1

                def d_hA(d, w=cnt["pe_sem"] - 1, k=k):
                    d.wait_ge(sem["pe_sem"], w)
                    d.tensor_tensor(out=ytA[:, :, :], in0=ps[1][:, :, :],
                                    in1=ap[:, :, :], op=Add)
                    d.drain()
                    d.tensor_scalar_mul(ytA[:, :, :], ytA[:, :, :],
                                        1.0 / k).then_inc(sem["dve_sem"], 1)
                dve_prog.append(d_hA)
                cnt["dve_sem"] += 1

                def d_hB(d, w=cnt["pe_sem"], k=k):
                    d.wait_ge(sem["pe_sem"], w)
                    d.tensor_tensor(out=ytB[:, :, :], in0=ps[2][:, :, :],
                                    in1=an[:, :, :], op=Add)
                    d.drain()
                    d.tensor_scalar_mul(ytB[:, :, :], ytB[:, :, :],
                                        1.0 / k).then_inc(sem["dve_sem"], 1)
                dve_prog.append(d_hB)
                cnt["dve_sem"] += 1
                first_h = False

            # P = I + ytA -> an buffer ; PT = I + ytB -> ap buffer
            def d_addI(d):
                d.drain()
                d.tensor_tensor(out=an[:, :, :], in0=ytA[:, :, :],
                                in1=identf[:, :, :], op=Add)
                d.drain()
                d.tensor_tensor(out=ap[:, :, :], in0=ytB[:, :, :],
                                in1=identf[:, :, :],
                                op=Add).then_inc(sem["dve_sem"], 1)
            dve_prog.append(d_addI)
            cnt["dve_sem"] += 1

            cur, nxt = (an, ap), (ytA, ytB)
            for s_i in range(EXPM_S):
                def p_sqA(t, w=cnt["dve_sem"], cur=cur):
                    t.wait_ge(sem["dve_sem"], w)
                    last = None
                    for mc in range(2):
                        for kc in range(2):
                            last = t.matmul(ps[1][:, mc, :],
                                            cur[1][:, kc, mc * P:(mc + 1) * P],
                                            cur[0][:, kc, :],
                                            start=(kc == 0), stop=(kc == 1))
                    last.then_inc(sem["pe_sem"], 1)
                pe_prog.append(p_sqA)
                cnt["pe_sem"] += 1

                def p_sqB(t, cur=cur):
                    last = None
                    for mc in range(2):
                        for kc in range(2):
                            last = t.matmul(ps[2][:, mc, :],
                                            cur[0][:, kc, mc * P:(mc + 1) * P],
                                            cur[1][:, kc, :],
                                            start=(kc == 0), stop=(kc == 1))
                    last.then_inc(sem["pe_sem"], 1)
                pe_prog.append(p_sqB)
                cnt["pe_sem"] += 1

                def d_sqA(d, w=cnt["pe_sem"] - 1, nxt=nxt):
                    d.wait_ge(sem["pe_sem"], w)
                    d.tensor_copy(nxt[0][:, :, :],
                                  ps[1][:, :, :]).then_inc(sem["dve_sem"], 1)
                dve_prog.append(d_sqA)
                cnt["dve_sem"] += 1

                def d_sqB(d, w=cnt["pe_sem"], nxt=nxt):
                    d.wait_ge(sem["pe_sem"], w)
                    d.tensor_copy(nxt[1][:, :, :],
                                  ps[2][:, :, :]).then_inc(sem["dve_sem"], 1)
                dve_prog.append(d_sqB)
                cnt["dve_sem"] += 1
                cur, nxt = nxt, cur

            def d_cast(d, b=b, cur=cur):
                d.drain()
                d.tensor_copy(pbf[:, b, :, :],
                              cur[0][:, :, :]).then_inc(sem["dve_sem"], 1)
            dve_prog.append(d_cast)
            cnt["dve_sem"] += 1
            entry_done[("p", b)] = ("dve_sem", cnt["dve_sem"])

        # ---------------- table build ----------------
        build_items = [("n", q) for q in range(2, 64)] + \
                      [("t", q) for q in range(64, 128)]
        bank_owner = {}
        entry_done[("n", 1)] = ("dve_sem", ident_done)
        entry_done[("t", 1)] = ("dve_sem", ident_done)
        pb_done = entry_done[("p", 1)][1]

        for j, (kind, q) in enumerate(build_items):
            bank = j % 8
            b = q & 1
            par = q >> 1

            waits = []
            if j == 0:
                waits.append(("dve_sem", pb_done))
            waits.append(entry_done[("n", par)])
            if bank in bank_owner:
                waits.append(bank_owner[bank])

            def p_build(t, kind=kind, b=b, par=par, bank=bank,
                        waits=tuple(waits)):
                for s_, c_ in waits:
                    t.wait_ge(sem[s_], c_)
                last = None
                for mc in range(2):
                    for kc in range(2):
                        if kind == "n":
                            lhsT = pbf[:, b, kc, mc * P:(mc + 1) * P]
                            rhs = ent3(rn, par)[:, kc, :]
                        else:
                            lhsT = ent3(rn, par)[:, kc, mc * P:(mc + 1) * P]
                            rhs = pbf[:, b, kc, :]
                        last = t.matmul(ps[bank][:, mc, :], lhsT, rhs,
                                        start=(kc == 0), stop=(kc == 1))
                last.then_inc(sem["pe_sem"], 1)
            pe_prog.append(p_build)
            cnt["pe_sem"] += 1

            ceng = "dve_sem" if j % 2 == 0 else "act_sem"
            prog = dve_prog if j % 2 == 0 else act_prog
            tab = rn if kind == "n" else rt

            def x_copy(e, tab=tab, q=q, bank=bank, w=cnt["pe_sem"], ceng=ceng):
                e.wait_ge(sem["pe_sem"], w)
                if ceng == "dve_sem":
                    e.tensor_copy(ent3(tab, q),
                                  ps[bank][:, :, :]).then_inc(sem[ceng], 1)
                else:
                    e.mul(ent3(tab, q),
                          ps[bank][:, :, :], 1.0).then_inc(sem[ceng], 1)
            prog.append(x_copy)
            cnt[ceng] += 1
            entry_done[(kind, q)] = (ceng, cnt[ceng])
            bank_owner[bank] = (ceng, cnt[ceng])

        build_dve = cnt["dve_sem"]
        build_act = cnt["act_sem"]

        # ---------------- positions ----------------
        def g_pos(g, bd=build_dve, ba=build_act, win=IN_TOTAL):
            g.wait_ge(sem["in_sem"], 16 * 7)  # all 7 input DMAs
            g.wait_ge(sem["dve_sem"], bd)
            g.wait_ge(sem["act_sem"], ba)
            with g.register("rgB") as rgB:
                for i in range(npos):
                    slot = i % NSTAGE_B
                    g.reg_load(rgB, offs_gp[i % P:i % P + 1, i // P:i // P + 1])
                    if i >= NSTAGE_B:
                        g.wait_ge(sem["mm1_sem"], i - NSTAGE_B + 1)
                    src = bass.AP(rn, rgB, [[NAT_STRIDE, P], [DIM, 2], [1, DIM]])
                    g.dma_start(stag_b[:, slot, :, :],
                                src).then_inc(sem[f"stg_s{(i % NSTAGE_B) // 2}"], 16)
        gps_prog.append(g_pos)

        def p_pos(t, bd=build_dve, ba=build_act):
            t.wait_ge(sem["dve_sem"], bd)
            t.wait_ge(sem["act_sem"], ba)
            NPAIRS_B = NSTAGE_B // 2
            npair = npos // 2
            with (t.register("rpA") as rpA, t.register("rloA") as rloA,
                  t.register("rhiA") as rhiA,
                  t.register("rpC") as rpC, t.register("rloC") as rloC,
                  t.register("rhiC") as rhiC):

                def mm1(i, r0, r1, mc, kc):
                    slot, bslot = i % NSTAGE, i % NSTAGE_B
                    rhs = bass.AP(rt, r0 if kc == 0 else r1,
                                  [[TRA_STRIDE, P], [1, DIM]])
                    ins = t.matmul(ps[slot][:, mc, :],
                                   stag_b[:, bslot, kc, mc * P:(mc + 1) * P],
                                   rhs, start=(kc == 0), stop=(kc == 1))
                    if mc == 1 and kc == 1:
                        ins.then_inc(sem["mm1_sem"], 1)

                def mm2(i, r0, r1, mc, kc):
                    slot = i % NSTAGE
                    rhs = bass.AP(rn, r0 if kc == 0 else r1,
                                  [[NAT_STRIDE, P], [1, DIM]])
                    ins = t.matmul(ps[4 + slot][:, mc, :],
                                   stag_x[:, i % NSX, kc, mc * P:(mc + 1) * P],
                                   rhs, start=(kc == 0), stop=(kc == 1))
                    if mc == 1 and kc == 1:
                        ins.then_inc(sem["mm2_sem"], 1)

                vals = {1: None, 2: None}

                def loads1(k):
                    pk, ck = k % P, k // P
                    t.reg_load(rpA, offs_a[pk:pk + 1, ck:ck + 1])
                    t.reg_alu(rloA, rpA, 65535, __import__("concourse").mybir.AluOpType.bitwise_and)
                    t.reg_alu(rhiA, rpA, 16, __import__("concourse").mybir.AluOpType.logical_shift_right)
                    v0, v1 = t.snap(rloA), t.snap(rhiA)
                    v0d, v1d = t.snap(v0 + DIM), t.snap(v1 + DIM)
                    vals[1] = (v0, v0d, v1, v1d)

                def waits1(k):
                    j = 2 * k + 1
                    t.wait_ge(sem[f"stg_s{k % NPAIRS_B}"],
                              32 * (k // NPAIRS_B + 1))
                    if j >= NSTAGE:
                        t.wait_ge(sem["dvex_sem"], j - NSTAGE + 1)

                def loads2(k):
                    pk, ck = k % P, k // P
                    t.reg_load(rpC, offs_c[pk:pk + 1, ck:ck + 1])
                    t.reg_alu(rloC, rpC, 65535, __import__("concourse").mybir.AluOpType.bitwise_and)
                    t.reg_alu(rhiC, rpC, 16, __import__("concourse").mybir.AluOpType.logical_shift_right)
                    v0, v1 = t.snap(rloC), t.snap(rhiC)
                    v0d, v1d = t.snap(v0 + DIM), t.snap(v1 + DIM)
                    vals[2] = (v0, v0d, v1, v1d)

                def waits2(k):
                    a, b = 2 * k, 2 * k + 1
                    if a >= NSTAGE:
                        t.wait_ge(sem["act_sem"], ba + b - NSTAGE + 1)
                    t.wait_ge(sem["dvex_sem"], b + 1)

                def seg_mms(kind, k):
                    v = vals[kind]
                    out = []
                    for (i, r0, r1) in ((2 * k, v[0], v[1]),
                                        (2 * k + 1, v[2], v[3])):
                        for mc in range(2):
                            for kc in range(2):
                                out.append((kind, i, r0, r1, mc, kc))
                    return out

                # segment order: P2 lags 2 pairs
                assert npair >= 2
                segs = [(1, 0), (1, 1)]
                for k in range(2, npair):
                    segs.append((2, k - 2))
                    segs.append((1, k))
                segs.append((2, npair - 2))
                segs.append((2, npair - 1))

                def emit_loads(kind, k):
                    (loads1 if kind == 1 else loads2)(k)

                def emit_waits(kind, k):
                    (waits1 if kind == 1 else waits2)(k)

                emit_loads(*segs[0])
                emit_waits(*segs[0])
                pre_done = [True] + [False] * (len(segs) - 1)
                for n, (kind, k) in enumerate(segs):
                    mms = seg_mms(kind, k)
                    # early injection only when the next segment uses the
                    # other register set (P1 vs P2)
                    inject = (n + 1 < len(segs)) and segs[n + 1][0] != kind
                    for idx, (kd, i, r0, r1, mc, kc) in enumerate(mms):
                        if idx == 2 and inject:
                            emit_loads(*segs[n + 1])
                        if idx == 6 and inject:
                            emit_waits(*segs[n + 1])
                            pre_done[n + 1] = True
                        (mm1 if kd == 1 else mm2)(i, r0, r1, mc, kc)
                    if (n + 1 < len(segs)) and not pre_done[n + 1]:
                        emit_loads(*segs[n + 1])
                        emit_waits(*segs[n + 1])
        pe_prog.append(p_pos)

        def d_pos(d):
            for i in range(npos):
                d.wait_ge(sem["mm1_sem"], i + 1)
                if i >= NSX:
                    d.wait_ge(sem["mm2_sem"], i - NSX + 1)
                d.tensor_copy(stag_x[:, i % NSX, :, :],
                              ps[i % NSTAGE][:, :, :]).then_inc(sem["dvex_sem"], 1)
        dve_prog.append(d_pos)

        def a_pos(a, ba=build_act):
            for i in range(npos):
                slot = i % NSTAGE
                oslot = i % NOUT
                a.wait_ge(sem["mm2_sem"], i + 1)
                if i >= NOUT:
                    # slot's previous DMAs done: all i//NOUT uses so far
                    a.wait_ge(sem[f"dma_s{oslot}"], 32 * (i // NOUT))
                a.mul(outb[:, oslot, :, :],
                      ps[4 + slot][:, :, :], 1.0).then_inc(sem["act_sem"], 1)
        act_prog.append(a_pos)

        def s_pos(s, ba=build_act):
            for i in range(npos):
                oslot = i % NOUT
                s.wait_ge(sem["act_sem"], ba + i + 1)
                for kc in range(2):
                    s.dma_start(out_ext[i, kc],
                                outb[:, oslot, kc, :]).then_inc(sem[f"dma_s{oslot}"], 16)
            for sl in range(NOUT):
                uses = len([i for i in range(npos) if i % NOUT == sl])
                if uses:
                    s.wait_ge(sem[f"dma_s{sl}"], 32 * uses)
        sync_prog.append(s_pos)

        # ---------------- emit ----------------
        with nc.Block() as block:
            @block.tensor
            def _(tensor):
                for fn in pe_prog:
                    fn(tensor)

            @block.vector
            def _(vector):
                for fn in dve_prog:
                    fn(vector)

            @block.scalar
            def _(scalar):
                for fn in act_prog:
                    fn(scalar)

            @block.gpsimd
            def _(gpsimd):
                for fn in gps_prog:
                    fn(gpsimd)

            @block.sync
            def _(sync):
                for fn in sync_prog:
                    fn(sync)

    return nc


def _host_offsets(u):
    """u: (n,) int64 positions -> (n,5) int32 element offsets
    [oB, oA0, oA1, oC0, oC1]."""
    u = u.astype(np.int64)
    blen = np.zeros_like(u)
    t = u.copy()
    while np.any(t > 0):
        blen = np.where(t > 0, blen + 1, blen)
        t >>= 1
    k = blen - 1  # path length
    tA = np.minimum(k, 6)
    idxA = (1 << tA) + (u & ((1 << tA) - 1))
    tB = np.clip(k - 6, 0, 5)
    idxB = (1 << tB) + ((u >> 6) & ((1 << tB) - 1))
    tC = np.clip(k - 11, 0, 5)
    idxC = (1 << tC) + ((u >> 11) & ((1 << tC) - 1))
    # short paths (p < 64): the whole product is a natural entry -> (1, p, 1)
    short = u < 64
    idxA = np.where(short, 1, idxA)
    idxB = np.where(short, u, idxB)
    assert idxA.max() < 128 and idxB.max() < 64 and idxC.max() < 64
    assert np.all((idxA == 1) | (idxA >= 64))
    oB = (idxB - 1) * ENT
    oA0 = np.where(idxA == 1, 0, (idxA - 63) * ENT)
    oC0 = (idxC - 1) * ENT
    return np.stack([oB, oA0, oA0 + DIM, oC0, oC0 + DIM], axis=1).astype(np.int32)


def kernel(primitives, identity, unique):
    global LAST_RESULTS
    from concourse.bass_utils import run_bass_kernel_spmd

    prims = np.ascontiguousarray(np.asarray(primitives, dtype=np.float32))
    u = np.asarray(unique).astype(np.int64).ravel()
    n = u.shape[0]
    assert n % NCORES == 0
    npos = n // NCORES

    offs5 = _host_offsets(u)  # (n, 5)

    if npos not in _NC_CACHE:
        nc = _build_nc(npos)
        nc.compile()
        _NC_CACHE[npos] = nc
    nc = _NC_CACHE[npos]

    npair = npos // 2
    ncol = (npos + P - 1) // P
    ncolp = (npair + P - 1) // P

    def pack_words(vals2):                       # (npair, 2) 16-bit -> [P, ncolp]
        w = (vals2[:, 0].astype(np.int64)
             | (vals2[:, 1].astype(np.int64) << 16)).astype(np.int32)
        q = np.zeros(ncolp * P, np.int32)
        q[:npair] = w
        return np.ascontiguousarray(q.reshape(ncolp, P).T)

    in_maps = []
    for c in range(NCORES):
        sl = offs5[c * npos:(c + 1) * npos]               # (npos, 5)
        a_arr = pack_words(sl[:, 1].reshape(npair, 2))
        c_arr = pack_words(sl[:, 3].reshape(npair, 2))
        gp_vals = np.zeros(ncol * P, np.int32)
        gp_vals[:npos] = sl[:, 0]
        gp_arr = np.ascontiguousarray(gp_vals.reshape(ncol, P).T)
        in_maps.append({"prims": prims, "offs_a": a_arr, "offs_c": c_arr,
                        "offs_gp": gp_arr})

    import os
    trace_dir = os.environ.get("KERNEL_TRACE_DIR")
    res = run_bass_kernel_spmd(nc, in_maps, core_ids=list(range(NCORES)),
                               tmpdir=trace_dir)
    LAST_RESULTS = res

    parts = []
    for c in range(NCORES):
        o = np.asarray(res.results[c]["out"])  # (npos, 2, 128, 256) f16
        parts.append(o.reshape(npos, DIM, DIM).astype(np.float32))
    out = np.concatenate(parts, axis=0)

    ident = np.asarray(identity, dtype=np.float32)[0]
    if not np.allclose(ident, np.eye(DIM, dtype=np.float32)):
        out = np.einsum("ij,njk->nik", ident, out).astype(np.float32)
    return out

